# revision 36
# baseline (speedup 1.0000x reference)
"""CapsuleLayer dynamic-routing kernel for 8 TRN2 NeuronCores (Bass/Tile).

Math restructure (u_hat is never materialized):
    u_hat[b,i,j,d] = sum_k x[b,i,k] W[i,j,k,d]
    s_r[b,(j,d)]   = X[b,(i,k)] @ (c_r odot W)[(i,k),(j,d)]      (matmul, K=(i,k))
    G[(i,k),(j,d)] = X^T @ v_r                                    (matmul, K=b)
    db[i,j]        = sum_{k,d} W[(i,k),(j,d)] * G[(i,k),(j,d)]    (DVE mult+reduce,
                     k-group partition sums via a block-ones matmul)

Sharding: input capsules I=1152 split 8 ways (144 per core). Bias/softmax are
core-local; each routing iteration all-reduces the s partials across the 8
cores; the last iteration reduce-scatters so each core squashes and emits its
own batch shard of v.

Perf notes (measured on HW, trace-driven):
  * collectives ship fp16 (CCE adds in fp16; eps 2^-11 keeps the added error
    ~0.1%) — halves the bounce DMAs and mesh wire bytes.
  * the FIRST ncfw collective of every execution pays a large entry barrier:
    its mesh begins at ~exec_start + 55-95us (run-variable), regardless of
    when it triggers. A dummy wake-up collective does NOT absorb it (tried:
    it only serializes its own mesh time in front of AR0); launch skew across
    the 8 PJRT dispatches (~12-18us) plus ~40-60us of ncfw barrier dominates.
    Later collectives enter in ~1.2us.
  * xt is packed batch-half-major so each s-matmul half waits on only one
    input DMA; wc0/xt split across both HWDGE queues, front of queue.
  * W is only shipped in bf16; the db product uses it directly (DVE mixes
    operand dtypes) — drops a 738KB f32 input DMA.
  * ones/dbr are bf16 so the db k-group-sum matmul is one PE pass (fp32
    lhsT would force the LOW_HIGH double-pass).
  * squash computes scale = exp(0.5*ln(s2+eps) - ln(1+s2)) — stays in the
    single ln/exp ACT table set and drops the reciprocal + one multiply
    from the DVE chain.
  * a remote_dma_broadcast hypercube exchange (bypassing ncfw and its entry
    barrier entirely) was attempted and faulted the device
    (NRT_EXEC_UNIT_UNRECOVERABLE 101) — cross-NC SBUF writes appear
    unsupported in this runtime configuration.
"""

import sys

sys.path.insert(0, "/opt/trn_rl_repo")

import numpy as np

import concourse.bacc as bacc
import concourse.bass as bass
import concourse.mybir as mybir
import concourse.tile as tile
from concourse.bass_utils import run_bass_kernel_spmd

F32 = mybir.dt.float32
F16 = mybir.dt.float16
BF16 = mybir.dt.bfloat16
AF = mybir.ActivationFunctionType
OP = mybir.AluOpType

B, I, DIN, J, D = 256, 1152, 8, 10, 16
NCORES = 8
IL = I // NCORES          # 144 input capsules per core
KI = IL * DIN             # 1152 local contraction length
NT = KI // 128            # 9 K-tiles of 128
JD = J * D                # 160
BL = B // NCORES          # 32 batch rows per core in the final scatter
NUM_ROUTING = 3
EPS = 1e-7

_ONE_ACT_SET = "natural_log_exp_and_others"


def _patch_act_tables():
    """Confine exp/ln to a single ACT table set so the table-load inserter
    emits exactly one load instead of thrashing (~1.3us per switch)."""
    orig = bacc.get_activation_tables
    if getattr(orig, "_capsule_patched", False):
        return

    def patched(arch):
        t = dict(orig(arch))
        return {k: (v if k == _ONE_ACT_SET else set()) for k, v in t.items()}

    patched._capsule_patched = True
    bacc.get_activation_tables = patched


def build():
    _patch_act_tables()
    nc = bacc.Bacc("TRN2", target_bir_lowering=False, debug=False,
                   num_devices=NCORES)

    # inputs are host-pre-tiled into the exact [128, *] SBUF layouts so each
    # array is one dense DMA (partition p holds row t*128+p of every tile t).
    # xt is batch-half-major: cols [m*KI + t*128 .. ] hold K-tile t of batch
    # half m, so the half-0 matmul chain waits on only the first xt DMA.
    xt_d = nc.dram_tensor("xt", [128, 2 * KI], BF16, kind="ExternalInput")
    w_d = nc.dram_tensor("w", [128, NT * JD], BF16, kind="ExternalInput")
    wc0_d = nc.dram_tensor("wc0", [128, NT * JD], BF16, kind="ExternalInput")
    id_d = nc.dram_tensor("ident", [128, 128], BF16, kind="ExternalInput")
    be_d = nc.dram_tensor("be", [128, NT * J], F32, kind="ExternalInput")
    ones_d = nc.dram_tensor("ones_blk", [128, 128], BF16, kind="ExternalInput")
    out_d = nc.dram_tensor("out", [BL, JD], F32, kind="ExternalOutput")

    groups = [list(range(NCORES))]

    with tile.TileContext(nc) as tc:
        with (
            tc.tile_pool(name="persist", bufs=1) as pp,
            tc.tile_pool(name="work", bufs=3) as wp,
            tc.tile_pool(name="spsum", bufs=2, space="PSUM") as sp,
            tc.tile_pool(name="gpsum", bufs=3, space="PSUM") as gp,
            tc.tile_pool(name="dbpsum", bufs=3, space="PSUM") as bp,
            tc.tile_pool(name="dram", bufs=1, space="DRAM") as dp,
        ):
            # NOTE (measured): the first ncfw collective's mesh start is
            # pinned at ~exec_start + 65-75us regardless of when it (or any
            # earlier dummy collective) triggers — a dummy wake-up collective
            # only serializes extra mesh time in front of AR0. Don't add one.

            # ---- persistent SBUF arrays ----
            xt_sb = pp.tile([128, 2 * KI], BF16, tag="xt")       # m-half major
            w_sb = pp.tile([128, NT * JD], BF16, tag="w")
            wc_sb = pp.tile([128, NT * JD], BF16, tag="wc")
            be_sb = pp.tile([128, NT * J], F32, tag="be")
            c_sb = pp.tile([128, NT * J], F32, tag="c")
            x2a_sb = pp.tile([128, KI], BF16, tag="x2a")
            x2b_sb = pp.tile([128, KI], BF16, tag="x2b")
            ones_sb = pp.tile([128, 128], BF16, tag="ones")
            id_sb = pp.tile([128, 128], BF16, tag="ident")
            eps_sb = pp.tile([128, 1], F32, tag="eps")
            nc.gpsimd.memset(eps_sb[:, :], EPS)
            one_sb = pp.tile([128, 1], F32, tag="one")
            nc.gpsimd.memset(one_sb[:, :], 1.0)
            sf_sb = pp.tile([128, 2 * JD], F16, tag="sf")        # full s, 2 b-tiles
            v_sb = pp.tile([128, 2 * JD], BF16, tag="v")
            dbr_sb = pp.tile([128, NT * J], BF16, tag="dbr")

            # ---- input DMAs. s0's critical path needs wc0 + xt: split them
            # across the two HWDGE queues, front of each queue. Everything
            # else (ident for the transposes at ~18us; be/ones/w not read
            # until iteration 1) trails on the SWDGE queue. ----
            # tiny leading DMAs for K-tile 0 so the first matmul can issue
            # ~1.5us before the bulk transfers land
            nc.sync.dma_start(out=wc_sb[:, :JD], in_=wc0_d[:, :JD])
            nc.scalar.dma_start(out=xt_sb[:, :128], in_=xt_d[:, :128])
            H0 = NT * JD // 2
            nc.sync.dma_start(out=wc_sb[:, JD:H0], in_=wc0_d[:, JD:H0])
            nc.scalar.dma_start(out=wc_sb[:, H0:], in_=wc0_d[:, H0:])
            nc.sync.dma_start(out=xt_sb[:, 128:KI], in_=xt_d[:, 128:KI])
            nc.scalar.dma_start(out=xt_sb[:, KI:], in_=xt_d[:, KI:])
            nc.gpsimd.dma_start(out=id_sb[:, :], in_=id_d[:, :])
            nc.gpsimd.dma_start(out=be_sb[:, :], in_=be_d[:, :])
            nc.gpsimd.dma_start(out=ones_sb[:, :], in_=ones_d[:, :])
            nc.gpsimd.dma_start(out=w_sb[:, :], in_=w_d[:, :])

            def softmax_q(q, r):
                """c_sb[:, q-third] = softmax(be_sb[:, q-third]) over each
                10-wide j segment. Per-q so the next matmul's Wc rebuild can
                pipeline behind each be update instead of waiting for all 9
                tiles. No max-subtraction: |b| stays well under exp overflow."""
                lo, hi = q * 3 * J, (q + 1) * 3 * J
                z = wp.tile([128, 3], F32, tag=f"z{q}", name=f"z_{r}_{q}")
                rz = wp.tile([128, 3], F32, tag=f"rz{q}", name=f"rz_{r}_{q}")
                nc.scalar.activation(out=c_sb[:, lo:hi], in_=be_sb[:, lo:hi],
                                     func=AF.Exp)
                nc.vector.tensor_reduce(
                    out=z[:, :],
                    in_=c_sb[:, lo:hi].rearrange("p (t j) -> p t j", t=3),
                    axis=mybir.AxisListType.X, op=OP.add)
                nc.vector.reciprocal(out=rz[:, :], in_=z[:, :])
                nc.vector.tensor_tensor(
                    out=c_sb[:, lo:hi].rearrange("p (t j) -> p t j", t=3),
                    in0=c_sb[:, lo:hi].rearrange("p (t j) -> p t j", t=3),
                    in1=rz.unsqueeze(2).broadcast_to([128, 3, J]),
                    op=OP.mult)

            def squash(s_ap, v_ap, np_, nt, wtag, sq_eng=None, ng=None):
                """v = squash(s) over d-segments; s_ap/v_ap are [np_, ng*D]
                (ng J-groups, default nt*J).

                scale = s2/(1+s2)/sqrt(s2+eps) is computed as
                exp(0.5*ln(s2+eps) - ln(1+s2)) — same ln/exp ACT table set,
                drops the reciprocal and one multiply from the DVE chain
                (error vs the eps-exact form is O(eps/s2), negligible)."""
                n = next(uid)
                if ng is None:
                    ng = nt * J
                s2 = wp.tile([128, ng], F32, tag=f"s2{wtag}",
                             name=f"s2_{n}")[:np_, :]
                aux = wp.tile([128, ng], F32, tag=f"aux{wtag}",
                              name=f"aux{n}")[:np_, :]
                scl = wp.tile([128, ng], F32, tag=f"scl{wtag}",
                              name=f"scl{n}")[:np_, :]
                sq = wp.tile([128, ng * D], F32, tag=f"sq{wtag}",
                             name=f"sq{n}")[:np_, :]
                # both halves' squashes serialize on the ACT queue (Square +
                # 2 Ln + Exp each); callers offload one half's Square to the
                # DVE via sq_eng to balance the two queues
                if sq_eng is None:
                    nc.scalar.activation(out=sq, in_=s_ap, func=AF.Square)
                else:
                    sq_eng.tensor_tensor(out=sq, in0=s_ap, in1=s_ap,
                                         op=OP.mult)
                nc.vector.tensor_reduce(
                    out=s2, in_=sq.rearrange("p (g d) -> p g d", d=D),
                    axis=mybir.AxisListType.X, op=OP.add)
                nc.scalar.activation(out=aux, in_=s2, func=AF.Ln,
                                     bias=eps_sb[:np_, :])        # ln(s2+eps)
                nc.scalar.activation(out=scl, in_=s2, func=AF.Ln,
                                     bias=one_sb[:np_, :])        # ln(1+s2)
                # scl = 0.5*ln(s2+eps) - ln(1+s2)
                nc.vector.scalar_tensor_tensor(out=scl, in0=aux, scalar=0.5,
                                               in1=scl, op0=OP.mult,
                                               op1=OP.subtract)
                nc.scalar.activation(out=scl, in_=scl, func=AF.Exp)
                nc.vector.tensor_tensor(
                    out=v_ap.rearrange("p (g d) -> p g d", d=D),
                    in0=s_ap.rearrange("p (g d) -> p g d", d=D),
                    in1=scl.unsqueeze(2).broadcast_to([np_, ng, D]),
                    op=OP.mult)

            uid = iter(range(10000))
            for r in range(NUM_ROUTING):
                last = r == NUM_ROUTING - 1
                s_ps = [sp.tile([128, JD], F32, tag="s_ps", name=f"s_ps_{r}_{m}")
                        for m in range(2)]
                cc_in = dp.tile([B, JD], F16, tag=f"cc_in{r}", name=f"cc_in{r}")
                s_stage = wp.tile([128, 2 * JD], F16, tag="s_stage",
                                  name=f"s_stage{r}")
                if r == 0:
                    # iteration 0 uses the host-precomputed wc0 already in
                    # wc_sb; batch-half-major so half 0 can stage and start
                    # its bounce DMA while half 1 is still on the PE
                    for m, eng in ((0, nc.sync), (1, nc.scalar)):
                        for t in range(NT):
                            nc.tensor.matmul(
                                s_ps[m][:, :],
                                lhsT=xt_sb[:, m * KI + t * 128:
                                           m * KI + (t + 1) * 128],
                                rhs=wc_sb[:, t * JD:(t + 1) * JD],
                                start=(t == 0), stop=(t == NT - 1))
                        nc.scalar.copy(out=s_stage[:, m * JD:(m + 1) * JD],
                                       in_=s_ps[m][:, :])
                        eng.dma_start(out=cc_in[m * 128:(m + 1) * 128, :],
                                      in_=s_stage[:, m * JD:(m + 1) * JD])
                else:
                    # later iterations pipeline per q-third: softmax(q) ->
                    # rebuild Wc(q) -> matmuls on q's 3 K-tiles, so the PE
                    # starts as soon as the first third of b is updated
                    for q in range(3):
                        softmax_q(q, r)
                        lo, hi = q * 3 * JD, (q + 1) * 3 * JD
                        eng = nc.gpsimd if q == 1 else nc.vector
                        eng.tensor_tensor(
                            out=wc_sb[:, lo:hi].rearrange("p (t j d) -> p t j d",
                                                          t=3, j=J),
                            in0=w_sb[:, lo:hi].rearrange("p (t j d) -> p t j d",
                                                         t=3, j=J),
                            in1=c_sb[:, q * 3 * J:(q + 1) * 3 * J]
                                .rearrange("p (t j) -> p t j", t=3)
                                .unsqueeze(3).broadcast_to([128, 3, J, D]),
                            op=OP.mult)
                        for m in range(2):
                            for t3 in range(3):
                                t = q * 3 + t3
                                nc.tensor.matmul(
                                    s_ps[m][:, :],
                                    lhsT=xt_sb[:, m * KI + t * 128:
                                               m * KI + (t + 1) * 128],
                                    rhs=wc_sb[:, t * JD:(t + 1) * JD],
                                    start=(t == 0), stop=(t == NT - 1))
                    for m, eng in ((0, nc.sync), (1, nc.scalar)):
                        nc.scalar.copy(out=s_stage[:, m * JD:(m + 1) * JD],
                                       in_=s_ps[m][:, :])
                        eng.dma_start(out=cc_in[m * 128:(m + 1) * 128, :],
                                      in_=s_stage[:, m * JD:(m + 1) * JD])
                if r == 0:
                    # build x2 (= xt^T) on device during the AR0 wait:
                    # PE transposes 128x128 blocks; DVE casts PSUM->SBUF bf16
                    for t in range(NT):
                        for m, dst in ((0, x2a_sb), (1, x2b_sb)):
                            t_ps = sp.tile([128, JD], BF16, tag="s_ps",
                                           name=f"t_ps_{t}_{m}")
                            nc.tensor.transpose(
                                t_ps[:, 0:128],
                                in_=xt_sb[:, m * KI + t * 128:
                                          m * KI + (t + 1) * 128],
                                identity=id_sb[:, :])
                            nc.vector.tensor_copy(
                                dst[:, t * 128:(t + 1) * 128], t_ps[:, 0:128])
                if not last:
                    cc_out = dp.tile([B, JD], F16, tag=f"cc_out{r}",
                                     name=f"cc_out{r}")
                    nc.gpsimd.collective_compute(
                        "AllReduce", OP.add, replica_groups=groups,
                        ins=[cc_in[:, :].opt()], outs=[cc_out[:, :].opt()])
                    # 4-way split of the result DMA: halves the wait before
                    # squash can start on batch half 0
                    H = JD // 2
                    for m in range(2):
                        for h, eng in ((0, nc.sync), (1, nc.scalar)):
                            eng.dma_start(
                                out=sf_sb[:, m * JD + h * H:
                                          m * JD + (h + 1) * H],
                                in_=cc_out[m * 128:(m + 1) * 128,
                                           h * H:(h + 1) * H])
                        squash(sf_sb[:, m * JD:(m + 1) * JD],
                               v_sb[:, m * JD:(m + 1) * JD], 128, 1, "f",
                               sq_eng=nc.vector if m == 1 else None)
                    # -- G = X^T @ v ; db rows; k-group sum; b += db --
                    # per-q ordering (pass a then b inside each q) so wg/db
                    # for q can start after 6 matmuls instead of 12. Only the
                    # first matmul per PSUM bank uses start=True (a later
                    # start would clear the whole bank's has_written bits and
                    # drop earlier sub-tiles); the rest rely on per-element
                    # overwrite.
                    for q in range(3):
                        g_ps = gp.tile([128, 3 * JD], F32, tag="g_ps",
                                       name=f"g_ps_{r}_{q}")
                        for t3 in range(3):
                            t = q * 3 + t3
                            nc.tensor.matmul(
                                g_ps[:, t3 * JD:(t3 + 1) * JD],
                                lhsT=x2a_sb[:, t * 128:(t + 1) * 128],
                                rhs=v_sb[:, 0:JD],
                                start=(t3 == 0), stop=False,
                                skip_group_check=True)
                        for t3 in range(3):
                            t = q * 3 + t3
                            nc.tensor.matmul(
                                g_ps[:, t3 * JD:(t3 + 1) * JD],
                                lhsT=x2b_sb[:, t * 128:(t + 1) * 128],
                                rhs=v_sb[:, JD:2 * JD],
                                start=False, stop=(t3 == 2),
                                skip_group_check=True)
                        # wg = W (bf16, mixed dtype) * G. The DVE saturates
                        # (~8us busy) if it owns the whole window, so split:
                        # q0 runs the short all-DVE chain (mult straight from
                        # PSUM); q1/q2 go ACT-stage -> gpsimd mult -> gpsimd
                        # reduce, entirely off the DVE.
                        wg = wp.tile([128, 3 * JD], F32, tag="wg",
                                     name=f"wg_{r}_{q}")
                        if q == 0:
                            nc.vector.tensor_tensor(
                                out=wg[:, :], in0=g_ps[:, :],
                                in1=w_sb[:, q * 3 * JD:(q + 1) * 3 * JD],
                                op=OP.mult)
                            red_eng = nc.vector
                        else:
                            g_sb = wp.tile([128, 3 * JD], F32, tag="g_sb",
                                           name=f"g_sb_{r}_{q}")
                            nc.scalar.copy(out=g_sb[:, :], in_=g_ps[:, :])
                            nc.gpsimd.tensor_tensor(
                                out=wg[:, :], in0=g_sb[:, :],
                                in1=w_sb[:, q * 3 * JD:(q + 1) * 3 * JD],
                                op=OP.mult)
                            red_eng = nc.vector
                        with nc.allow_low_precision(
                                reason="db logits tolerate bf16 rounding"):
                            red_eng.tensor_reduce(
                                out=dbr_sb[:, q * 3 * J:(q + 1) * 3 * J],
                                in_=wg.rearrange("p (g d) -> p g d", d=D),
                                axis=mybir.AxisListType.X, op=OP.add)
                        # bf16 ones/dbr: fp32 lhsT would force the PE's
                        # LOW_HIGH double-pass (~0.8us/q); bf16 is one pass
                        db_ps = bp.tile([128, 3 * J], F32, tag="db_ps",
                                        name=f"db_ps{r}_{q}")
                        nc.tensor.matmul(db_ps[:, :], lhsT=ones_sb[:, :],
                                         rhs=dbr_sb[:, q * 3 * J:(q + 1) * 3 * J],
                                         start=True, stop=True)
                        nc.vector.tensor_tensor(
                            out=be_sb[:, q * 3 * J:(q + 1) * 3 * J],
                            in0=be_sb[:, q * 3 * J:(q + 1) * 3 * J],
                            in1=db_ps[:, :], op=OP.add)
                else:
                    rs_out = dp.tile([BL, JD], F16, tag="rs_out", name="rs_out")
                    nc.gpsimd.collective_compute(
                        "ReduceScatter", OP.add, replica_groups=groups,
                        ins=[cc_in[:, :].opt()], outs=[rs_out[:, :].opt()])
                    s_loc = wp.tile([128, JD], F16, tag="s_loc",
                                    name="s_loc")[:BL, :]
                    v_loc = wp.tile([128, JD], F32, tag="v_loc",
                                    name="v_loc")[:BL, :]
                    # tail pipelined in two J-group chunks: chunk 0's squash
                    # runs while chunk 1's rs_out DMA is still landing, and
                    # chunk 0's output DMA overlaps chunk 1's squash
                    Hc = JD // 2
                    for h, eng in ((0, nc.sync), (1, nc.scalar)):
                        eng.dma_start(out=s_loc[:, h * Hc:(h + 1) * Hc],
                                      in_=rs_out[:, h * Hc:(h + 1) * Hc])
                        squash(s_loc[:, h * Hc:(h + 1) * Hc],
                               v_loc[:, h * Hc:(h + 1) * Hc], BL, 1, f"l{h}",
                               sq_eng=nc.vector if h == 1 else None,
                               ng=Hc // D)
                        eng.dma_start(out=out_d[:, h * Hc:(h + 1) * Hc],
                                      in_=v_loc[:, h * Hc:(h + 1) * Hc])

    nc.compile()
    return nc


_CACHE = {}


def _get_nc():
    if "nc" not in _CACHE:
        _CACHE["nc"] = build()
    return _CACHE["nc"]


def _prep_inputs(inputs, W, bias):
    import ml_dtypes
    bf16 = ml_dtypes.bfloat16

    inputs = np.ascontiguousarray(inputs, dtype=np.float32)
    W4 = np.ascontiguousarray(W, dtype=np.float32).reshape(I, J, DIN, D)
    bias = np.ascontiguousarray(bias, dtype=np.float32)
    ones_blk = np.zeros((128, 128), dtype=np.float32)
    for g in range(16):
        ones_blk[g * 8:(g + 1) * 8, g * 8:(g + 1) * 8] = 1.0
    ones_blk = ones_blk.astype(bf16)

    def pack(a):
        """[KI, F] -> [128, NT*F]: partition p holds row t*128+p of tile t."""
        f = a.shape[1]
        return np.ascontiguousarray(
            a.reshape(NT, 128, f).transpose(1, 0, 2).reshape(128, NT * f))

    ident = np.eye(128, dtype=bf16)
    in_maps = []
    for r in range(NCORES):
        xl = inputs[:, r * IL:(r + 1) * IL, :]                    # [B, IL, DIN]
        xt = pack(xl.transpose(1, 2, 0).reshape(KI, B)).astype(bf16)
        # batch-half-major: [128, 2*KI], half m holds its 9 K-tiles contiguous
        xt = np.ascontiguousarray(
            xt.reshape(128, NT, 2, 128).transpose(0, 2, 1, 3).reshape(128, 2 * KI))
        w2 = W4[r * IL:(r + 1) * IL].transpose(0, 2, 1, 3).reshape(KI, JD)
        bl = bias[r * IL:(r + 1) * IL, :]
        e = np.exp(bl - bl.max(axis=1, keepdims=True))
        c0 = e / e.sum(axis=1, keepdims=True)                     # [IL, J]
        c0e = np.repeat(c0, DIN, axis=0)[:, :, None]              # [(i k), J, 1]
        wc0 = pack((w2.reshape(KI, J, D) * c0e).reshape(KI, JD)).astype(bf16)
        wl = pack(w2).astype(bf16)
        be = pack(np.repeat(bias[r * IL:(r + 1) * IL, :], DIN, axis=0))
        in_maps.append({"xt": xt, "w": wl, "wc0": wc0,
                        "ident": ident, "be": be, "ones_blk": ones_blk})
    return in_maps


def run(inputs, W, bias, trace=False, **spmd_kwargs):
    nc = _get_nc()
    in_maps = _prep_inputs(inputs, W, bias)
    res = run_bass_kernel_spmd(nc, in_maps, list(range(NCORES)),
                               trace=trace, **spmd_kwargs)
    v = np.concatenate([res.results[r]["out"] for r in range(NCORES)], axis=0)
    return v.reshape(B, J, D).astype(np.float32), res


def kernel(inputs, W, bias):
    out, _ = run(inputs, W, bias, trace=False)
    return out



# revision 37
# speedup vs baseline: 1.0111x; 1.0111x over previous
"""CapsuleLayer dynamic-routing kernel for 8 TRN2 NeuronCores (Bass/Tile).

Math restructure (u_hat is never materialized):
    u_hat[b,i,j,d] = sum_k x[b,i,k] W[i,j,k,d]
    s_r[b,(j,d)]   = X[b,(i,k)] @ (c_r odot W)[(i,k),(j,d)]      (matmul, K=(i,k))
    G[(i,k),(j,d)] = X^T @ v_r                                    (matmul, K=b)
    db[i,j]        = sum_{k,d} W[(i,k),(j,d)] * G[(i,k),(j,d)]    (DVE mult+reduce,
                     k-group partition sums via a block-ones matmul)

Sharding: input capsules I=1152 split 8 ways (144 per core). Bias/softmax are
core-local; each routing iteration all-reduces the s partials across the 8
cores; the last iteration reduce-scatters so each core squashes and emits its
own batch shard of v.

Perf notes (measured on HW, trace-driven):
  * collectives ship fp16 (CCE adds in fp16; eps 2^-11 keeps the added error
    ~0.1%) — halves the bounce DMAs and mesh wire bytes.
  * the FIRST ncfw collective of every execution pays a large entry barrier:
    its mesh begins at ~exec_start + 55-95us (run-variable), regardless of
    when it triggers. A dummy wake-up collective does NOT absorb it (tried:
    it only serializes its own mesh time in front of AR0); launch skew across
    the 8 PJRT dispatches (~12-18us) plus ~40-60us of ncfw barrier dominates.
    Later collectives enter in ~1.2us.
  * xt is packed batch-half-major so each s-matmul half waits on only one
    input DMA; wc0/xt split across both HWDGE queues, front of queue.
  * W is only shipped in bf16; the db product uses it directly (DVE mixes
    operand dtypes) — drops a 738KB f32 input DMA.
  * ones/dbr are bf16 so the db k-group-sum matmul is one PE pass (fp32
    lhsT would force the LOW_HIGH double-pass).
  * squash computes scale = exp(0.5*ln(s2+eps) - ln(1+s2)) — stays in the
    single ln/exp ACT table set and drops the reciprocal + one multiply
    from the DVE chain.
  * a remote_dma_broadcast hypercube exchange (bypassing ncfw and its entry
    barrier entirely) was attempted and faulted the device
    (NRT_EXEC_UNIT_UNRECOVERABLE 101) — cross-NC SBUF writes appear
    unsupported in this runtime configuration.
"""

import os
import sys

sys.path.insert(0, "/opt/trn_rl_repo")

# Force the Mesh collective algorithm: the runtime picks RDH for our 80KB
# AllReduces (3 stages x 2 ncfw software passes each ~= 11.4-15us); Mesh is
# one hop. Must be set before the Neuron runtime loads.
os.environ.setdefault("NEURON_RT_DBG_RDH_CC", "0")

import numpy as np

import concourse.bacc as bacc
import concourse.bass as bass
import concourse.mybir as mybir
import concourse.tile as tile
from concourse.bass_utils import run_bass_kernel_spmd

F32 = mybir.dt.float32
F16 = mybir.dt.float16
BF16 = mybir.dt.bfloat16
AF = mybir.ActivationFunctionType
OP = mybir.AluOpType

B, I, DIN, J, D = 256, 1152, 8, 10, 16
NCORES = 8
IL = I // NCORES          # 144 input capsules per core
KI = IL * DIN             # 1152 local contraction length
NT = KI // 128            # 9 K-tiles of 128
JD = J * D                # 160
BL = B // NCORES          # 32 batch rows per core in the final scatter
NUM_ROUTING = 3
EPS = 1e-7

_ONE_ACT_SET = "natural_log_exp_and_others"


def _patch_act_tables():
    """Confine exp/ln to a single ACT table set so the table-load inserter
    emits exactly one load instead of thrashing (~1.3us per switch)."""
    orig = bacc.get_activation_tables
    if getattr(orig, "_capsule_patched", False):
        return

    def patched(arch):
        t = dict(orig(arch))
        return {k: (v if k == _ONE_ACT_SET else set()) for k, v in t.items()}

    patched._capsule_patched = True
    bacc.get_activation_tables = patched


def build():
    _patch_act_tables()
    nc = bacc.Bacc("TRN2", target_bir_lowering=False, debug=False,
                   num_devices=NCORES)

    # inputs are host-pre-tiled into the exact [128, *] SBUF layouts so each
    # array is one dense DMA (partition p holds row t*128+p of every tile t).
    # xt is batch-half-major: cols [m*KI + t*128 .. ] hold K-tile t of batch
    # half m, so the half-0 matmul chain waits on only the first xt DMA.
    xt_d = nc.dram_tensor("xt", [128, 2 * KI], BF16, kind="ExternalInput")
    w_d = nc.dram_tensor("w", [128, NT * JD], BF16, kind="ExternalInput")
    wc0_d = nc.dram_tensor("wc0", [128, NT * JD], BF16, kind="ExternalInput")
    id_d = nc.dram_tensor("ident", [128, 128], BF16, kind="ExternalInput")
    be_d = nc.dram_tensor("be", [128, NT * J], F32, kind="ExternalInput")
    ones_d = nc.dram_tensor("ones_blk", [128, 128], BF16, kind="ExternalInput")
    out_d = nc.dram_tensor("out", [BL, JD], F32, kind="ExternalOutput")

    groups = [list(range(NCORES))]

    with tile.TileContext(nc) as tc:
        with (
            tc.tile_pool(name="persist", bufs=1) as pp,
            tc.tile_pool(name="work", bufs=3) as wp,
            tc.tile_pool(name="spsum", bufs=2, space="PSUM") as sp,
            tc.tile_pool(name="gpsum", bufs=3, space="PSUM") as gp,
            tc.tile_pool(name="dbpsum", bufs=3, space="PSUM") as bp,
            tc.tile_pool(name="dram", bufs=1, space="DRAM") as dp,
        ):
            # NOTE (measured): the first ncfw collective's mesh start is
            # pinned at ~exec_start + 65-75us regardless of when it (or any
            # earlier dummy collective) triggers — a dummy wake-up collective
            # only serializes extra mesh time in front of AR0. Don't add one.

            # ---- persistent SBUF arrays ----
            xt_sb = pp.tile([128, 2 * KI], BF16, tag="xt")       # m-half major
            w_sb = pp.tile([128, NT * JD], BF16, tag="w")
            wc_sb = pp.tile([128, NT * JD], BF16, tag="wc")
            be_sb = pp.tile([128, NT * J], F32, tag="be")
            c_sb = pp.tile([128, NT * J], F32, tag="c")
            x2a_sb = pp.tile([128, KI], BF16, tag="x2a")
            x2b_sb = pp.tile([128, KI], BF16, tag="x2b")
            ones_sb = pp.tile([128, 128], BF16, tag="ones")
            id_sb = pp.tile([128, 128], BF16, tag="ident")
            eps_sb = pp.tile([128, 1], F32, tag="eps")
            nc.gpsimd.memset(eps_sb[:, :], EPS)
            one_sb = pp.tile([128, 1], F32, tag="one")
            nc.gpsimd.memset(one_sb[:, :], 1.0)
            sf_sb = pp.tile([128, 2 * JD], F16, tag="sf")        # full s, 2 b-tiles
            v_sb = pp.tile([128, 2 * JD], BF16, tag="v")
            dbr_sb = pp.tile([128, NT * J], BF16, tag="dbr")

            # ---- input DMAs. s0's critical path needs wc0 + xt: split them
            # across the two HWDGE queues, front of each queue. Everything
            # else (ident for the transposes at ~18us; be/ones/w not read
            # until iteration 1) trails on the SWDGE queue. ----
            # tiny leading DMAs for K-tile 0 so the first matmul can issue
            # ~1.5us before the bulk transfers land
            nc.sync.dma_start(out=wc_sb[:, :JD], in_=wc0_d[:, :JD])
            nc.scalar.dma_start(out=xt_sb[:, :128], in_=xt_d[:, :128])
            H0 = NT * JD // 2
            nc.sync.dma_start(out=wc_sb[:, JD:H0], in_=wc0_d[:, JD:H0])
            nc.scalar.dma_start(out=wc_sb[:, H0:], in_=wc0_d[:, H0:])
            nc.sync.dma_start(out=xt_sb[:, 128:KI], in_=xt_d[:, 128:KI])
            nc.scalar.dma_start(out=xt_sb[:, KI:], in_=xt_d[:, KI:])
            nc.gpsimd.dma_start(out=id_sb[:, :], in_=id_d[:, :])
            nc.gpsimd.dma_start(out=be_sb[:, :], in_=be_d[:, :])
            nc.gpsimd.dma_start(out=ones_sb[:, :], in_=ones_d[:, :])
            nc.gpsimd.dma_start(out=w_sb[:, :], in_=w_d[:, :])

            def softmax_q(q, r):
                """c_sb[:, q-third] = softmax(be_sb[:, q-third]) over each
                10-wide j segment. Per-q so the next matmul's Wc rebuild can
                pipeline behind each be update instead of waiting for all 9
                tiles. No max-subtraction: |b| stays well under exp overflow."""
                lo, hi = q * 3 * J, (q + 1) * 3 * J
                z = wp.tile([128, 3], F32, tag=f"z{q}", name=f"z_{r}_{q}")
                rz = wp.tile([128, 3], F32, tag=f"rz{q}", name=f"rz_{r}_{q}")
                nc.scalar.activation(out=c_sb[:, lo:hi], in_=be_sb[:, lo:hi],
                                     func=AF.Exp)
                nc.vector.tensor_reduce(
                    out=z[:, :],
                    in_=c_sb[:, lo:hi].rearrange("p (t j) -> p t j", t=3),
                    axis=mybir.AxisListType.X, op=OP.add)
                nc.vector.reciprocal(out=rz[:, :], in_=z[:, :])
                nc.vector.tensor_tensor(
                    out=c_sb[:, lo:hi].rearrange("p (t j) -> p t j", t=3),
                    in0=c_sb[:, lo:hi].rearrange("p (t j) -> p t j", t=3),
                    in1=rz.unsqueeze(2).broadcast_to([128, 3, J]),
                    op=OP.mult)

            def squash(s_ap, v_ap, np_, nt, wtag, sq_eng=None, ng=None):
                """v = squash(s) over d-segments; s_ap/v_ap are [np_, ng*D]
                (ng J-groups, default nt*J).

                scale = s2/(1+s2)/sqrt(s2+eps) is computed as
                exp(0.5*ln(s2+eps) - ln(1+s2)) — same ln/exp ACT table set,
                drops the reciprocal and one multiply from the DVE chain
                (error vs the eps-exact form is O(eps/s2), negligible)."""
                n = next(uid)
                if ng is None:
                    ng = nt * J
                s2 = wp.tile([128, ng], F32, tag=f"s2{wtag}",
                             name=f"s2_{n}")[:np_, :]
                aux = wp.tile([128, ng], F32, tag=f"aux{wtag}",
                              name=f"aux{n}")[:np_, :]
                scl = wp.tile([128, ng], F32, tag=f"scl{wtag}",
                              name=f"scl{n}")[:np_, :]
                sq = wp.tile([128, ng * D], F32, tag=f"sq{wtag}",
                             name=f"sq{n}")[:np_, :]
                # both halves' squashes serialize on the ACT queue (Square +
                # 2 Ln + Exp each); callers offload one half's Square to the
                # DVE via sq_eng to balance the two queues
                if sq_eng is None:
                    nc.scalar.activation(out=sq, in_=s_ap, func=AF.Square)
                else:
                    sq_eng.tensor_tensor(out=sq, in0=s_ap, in1=s_ap,
                                         op=OP.mult)
                nc.vector.tensor_reduce(
                    out=s2, in_=sq.rearrange("p (g d) -> p g d", d=D),
                    axis=mybir.AxisListType.X, op=OP.add)
                nc.scalar.activation(out=aux, in_=s2, func=AF.Ln,
                                     bias=eps_sb[:np_, :])        # ln(s2+eps)
                nc.scalar.activation(out=scl, in_=s2, func=AF.Ln,
                                     bias=one_sb[:np_, :])        # ln(1+s2)
                # scl = 0.5*ln(s2+eps) - ln(1+s2)
                nc.vector.scalar_tensor_tensor(out=scl, in0=aux, scalar=0.5,
                                               in1=scl, op0=OP.mult,
                                               op1=OP.subtract)
                nc.scalar.activation(out=scl, in_=scl, func=AF.Exp)
                nc.vector.tensor_tensor(
                    out=v_ap.rearrange("p (g d) -> p g d", d=D),
                    in0=s_ap.rearrange("p (g d) -> p g d", d=D),
                    in1=scl.unsqueeze(2).broadcast_to([np_, ng, D]),
                    op=OP.mult)

            uid = iter(range(10000))
            for r in range(NUM_ROUTING):
                last = r == NUM_ROUTING - 1
                s_ps = [sp.tile([128, JD], F32, tag="s_ps", name=f"s_ps_{r}_{m}")
                        for m in range(2)]
                cc_in = dp.tile([B, JD], F16, tag=f"cc_in{r}", name=f"cc_in{r}")
                s_stage = wp.tile([128, 2 * JD], F16, tag="s_stage",
                                  name=f"s_stage{r}")
                if r == 0:
                    # iteration 0 uses the host-precomputed wc0 already in
                    # wc_sb; batch-half-major so half 0 can stage and start
                    # its bounce DMA while half 1 is still on the PE
                    for m, eng in ((0, nc.sync), (1, nc.scalar)):
                        for t in range(NT):
                            nc.tensor.matmul(
                                s_ps[m][:, :],
                                lhsT=xt_sb[:, m * KI + t * 128:
                                           m * KI + (t + 1) * 128],
                                rhs=wc_sb[:, t * JD:(t + 1) * JD],
                                start=(t == 0), stop=(t == NT - 1))
                        nc.scalar.copy(out=s_stage[:, m * JD:(m + 1) * JD],
                                       in_=s_ps[m][:, :])
                        eng.dma_start(out=cc_in[m * 128:(m + 1) * 128, :],
                                      in_=s_stage[:, m * JD:(m + 1) * JD])
                else:
                    # later iterations pipeline per q-third: softmax(q) ->
                    # rebuild Wc(q) -> matmuls on q's 3 K-tiles, so the PE
                    # starts as soon as the first third of b is updated
                    for q in range(3):
                        softmax_q(q, r)
                        lo, hi = q * 3 * JD, (q + 1) * 3 * JD
                        eng = nc.gpsimd if q == 1 else nc.vector
                        eng.tensor_tensor(
                            out=wc_sb[:, lo:hi].rearrange("p (t j d) -> p t j d",
                                                          t=3, j=J),
                            in0=w_sb[:, lo:hi].rearrange("p (t j d) -> p t j d",
                                                         t=3, j=J),
                            in1=c_sb[:, q * 3 * J:(q + 1) * 3 * J]
                                .rearrange("p (t j) -> p t j", t=3)
                                .unsqueeze(3).broadcast_to([128, 3, J, D]),
                            op=OP.mult)
                        for m in range(2):
                            for t3 in range(3):
                                t = q * 3 + t3
                                nc.tensor.matmul(
                                    s_ps[m][:, :],
                                    lhsT=xt_sb[:, m * KI + t * 128:
                                               m * KI + (t + 1) * 128],
                                    rhs=wc_sb[:, t * JD:(t + 1) * JD],
                                    start=(t == 0), stop=(t == NT - 1))
                    for m, eng in ((0, nc.sync), (1, nc.scalar)):
                        nc.scalar.copy(out=s_stage[:, m * JD:(m + 1) * JD],
                                       in_=s_ps[m][:, :])
                        eng.dma_start(out=cc_in[m * 128:(m + 1) * 128, :],
                                      in_=s_stage[:, m * JD:(m + 1) * JD])
                if r == 0:
                    # build x2 (= xt^T) on device during the AR0 wait:
                    # PE transposes 128x128 blocks; DVE casts PSUM->SBUF bf16
                    for t in range(NT):
                        for m, dst in ((0, x2a_sb), (1, x2b_sb)):
                            t_ps = sp.tile([128, JD], BF16, tag="s_ps",
                                           name=f"t_ps_{t}_{m}")
                            nc.tensor.transpose(
                                t_ps[:, 0:128],
                                in_=xt_sb[:, m * KI + t * 128:
                                          m * KI + (t + 1) * 128],
                                identity=id_sb[:, :])
                            nc.vector.tensor_copy(
                                dst[:, t * 128:(t + 1) * 128], t_ps[:, 0:128])
                if not last:
                    cc_out = dp.tile([B, JD], F16, tag=f"cc_out{r}",
                                     name=f"cc_out{r}")
                    nc.gpsimd.collective_compute(
                        "AllReduce", OP.add, replica_groups=groups,
                        ins=[cc_in[:, :].opt()], outs=[cc_out[:, :].opt()])
                    # 4-way split of the result DMA: halves the wait before
                    # squash can start on batch half 0
                    H = JD // 2
                    for m in range(2):
                        for h, eng in ((0, nc.sync), (1, nc.scalar)):
                            eng.dma_start(
                                out=sf_sb[:, m * JD + h * H:
                                          m * JD + (h + 1) * H],
                                in_=cc_out[m * 128:(m + 1) * 128,
                                           h * H:(h + 1) * H])
                        squash(sf_sb[:, m * JD:(m + 1) * JD],
                               v_sb[:, m * JD:(m + 1) * JD], 128, 1, "f",
                               sq_eng=nc.vector if m == 1 else None)
                    # -- G = X^T @ v ; db rows; k-group sum; b += db --
                    # per-q ordering (pass a then b inside each q) so wg/db
                    # for q can start after 6 matmuls instead of 12. Only the
                    # first matmul per PSUM bank uses start=True (a later
                    # start would clear the whole bank's has_written bits and
                    # drop earlier sub-tiles); the rest rely on per-element
                    # overwrite.
                    for q in range(3):
                        g_ps = gp.tile([128, 3 * JD], F32, tag="g_ps",
                                       name=f"g_ps_{r}_{q}")
                        for t3 in range(3):
                            t = q * 3 + t3
                            nc.tensor.matmul(
                                g_ps[:, t3 * JD:(t3 + 1) * JD],
                                lhsT=x2a_sb[:, t * 128:(t + 1) * 128],
                                rhs=v_sb[:, 0:JD],
                                start=(t3 == 0), stop=False,
                                skip_group_check=True)
                        for t3 in range(3):
                            t = q * 3 + t3
                            nc.tensor.matmul(
                                g_ps[:, t3 * JD:(t3 + 1) * JD],
                                lhsT=x2b_sb[:, t * 128:(t + 1) * 128],
                                rhs=v_sb[:, JD:2 * JD],
                                start=False, stop=(t3 == 2),
                                skip_group_check=True)
                        # wg = W (bf16, mixed dtype) * G. The DVE saturates
                        # (~8us busy) if it owns the whole window, so split:
                        # q0 runs the short all-DVE chain (mult straight from
                        # PSUM); q1/q2 go ACT-stage -> gpsimd mult -> gpsimd
                        # reduce, entirely off the DVE.
                        wg = wp.tile([128, 3 * JD], F32, tag="wg",
                                     name=f"wg_{r}_{q}")
                        if q == 0:
                            nc.vector.tensor_tensor(
                                out=wg[:, :], in0=g_ps[:, :],
                                in1=w_sb[:, q * 3 * JD:(q + 1) * 3 * JD],
                                op=OP.mult)
                            red_eng = nc.vector
                        else:
                            g_sb = wp.tile([128, 3 * JD], F32, tag="g_sb",
                                           name=f"g_sb_{r}_{q}")
                            nc.scalar.copy(out=g_sb[:, :], in_=g_ps[:, :])
                            nc.gpsimd.tensor_tensor(
                                out=wg[:, :], in0=g_sb[:, :],
                                in1=w_sb[:, q * 3 * JD:(q + 1) * 3 * JD],
                                op=OP.mult)
                            red_eng = nc.vector
                        with nc.allow_low_precision(
                                reason="db logits tolerate bf16 rounding"):
                            red_eng.tensor_reduce(
                                out=dbr_sb[:, q * 3 * J:(q + 1) * 3 * J],
                                in_=wg.rearrange("p (g d) -> p g d", d=D),
                                axis=mybir.AxisListType.X, op=OP.add)
                        # bf16 ones/dbr: fp32 lhsT would force the PE's
                        # LOW_HIGH double-pass (~0.8us/q); bf16 is one pass
                        db_ps = bp.tile([128, 3 * J], F32, tag="db_ps",
                                        name=f"db_ps{r}_{q}")
                        nc.tensor.matmul(db_ps[:, :], lhsT=ones_sb[:, :],
                                         rhs=dbr_sb[:, q * 3 * J:(q + 1) * 3 * J],
                                         start=True, stop=True)
                        nc.vector.tensor_tensor(
                            out=be_sb[:, q * 3 * J:(q + 1) * 3 * J],
                            in0=be_sb[:, q * 3 * J:(q + 1) * 3 * J],
                            in1=db_ps[:, :], op=OP.add)
                else:
                    rs_out = dp.tile([BL, JD], F16, tag="rs_out", name="rs_out")
                    nc.gpsimd.collective_compute(
                        "ReduceScatter", OP.add, replica_groups=groups,
                        ins=[cc_in[:, :].opt()], outs=[rs_out[:, :].opt()])
                    s_loc = wp.tile([128, JD], F16, tag="s_loc",
                                    name="s_loc")[:BL, :]
                    v_loc = wp.tile([128, JD], F32, tag="v_loc",
                                    name="v_loc")[:BL, :]
                    # tail pipelined in two J-group chunks: chunk 0's squash
                    # runs while chunk 1's rs_out DMA is still landing, and
                    # chunk 0's output DMA overlaps chunk 1's squash
                    Hc = JD // 2
                    for h, eng in ((0, nc.sync), (1, nc.scalar)):
                        eng.dma_start(out=s_loc[:, h * Hc:(h + 1) * Hc],
                                      in_=rs_out[:, h * Hc:(h + 1) * Hc])
                        squash(s_loc[:, h * Hc:(h + 1) * Hc],
                               v_loc[:, h * Hc:(h + 1) * Hc], BL, 1, f"l{h}",
                               sq_eng=nc.vector if h == 1 else None,
                               ng=Hc // D)
                        eng.dma_start(out=out_d[:, h * Hc:(h + 1) * Hc],
                                      in_=v_loc[:, h * Hc:(h + 1) * Hc])

    nc.compile()
    return nc


_CACHE = {}


def _get_nc():
    if "nc" not in _CACHE:
        _CACHE["nc"] = build()
    return _CACHE["nc"]


def _prep_inputs(inputs, W, bias):
    import ml_dtypes
    bf16 = ml_dtypes.bfloat16

    inputs = np.ascontiguousarray(inputs, dtype=np.float32)
    W4 = np.ascontiguousarray(W, dtype=np.float32).reshape(I, J, DIN, D)
    bias = np.ascontiguousarray(bias, dtype=np.float32)
    ones_blk = np.zeros((128, 128), dtype=np.float32)
    for g in range(16):
        ones_blk[g * 8:(g + 1) * 8, g * 8:(g + 1) * 8] = 1.0
    ones_blk = ones_blk.astype(bf16)

    def pack(a):
        """[KI, F] -> [128, NT*F]: partition p holds row t*128+p of tile t."""
        f = a.shape[1]
        return np.ascontiguousarray(
            a.reshape(NT, 128, f).transpose(1, 0, 2).reshape(128, NT * f))

    ident = np.eye(128, dtype=bf16)
    in_maps = []
    for r in range(NCORES):
        xl = inputs[:, r * IL:(r + 1) * IL, :]                    # [B, IL, DIN]
        xt = pack(xl.transpose(1, 2, 0).reshape(KI, B)).astype(bf16)
        # batch-half-major: [128, 2*KI], half m holds its 9 K-tiles contiguous
        xt = np.ascontiguousarray(
            xt.reshape(128, NT, 2, 128).transpose(0, 2, 1, 3).reshape(128, 2 * KI))
        w2 = W4[r * IL:(r + 1) * IL].transpose(0, 2, 1, 3).reshape(KI, JD)
        bl = bias[r * IL:(r + 1) * IL, :]
        e = np.exp(bl - bl.max(axis=1, keepdims=True))
        c0 = e / e.sum(axis=1, keepdims=True)                     # [IL, J]
        c0e = np.repeat(c0, DIN, axis=0)[:, :, None]              # [(i k), J, 1]
        wc0 = pack((w2.reshape(KI, J, D) * c0e).reshape(KI, JD)).astype(bf16)
        wl = pack(w2).astype(bf16)
        be = pack(np.repeat(bias[r * IL:(r + 1) * IL, :], DIN, axis=0))
        in_maps.append({"xt": xt, "w": wl, "wc0": wc0,
                        "ident": ident, "be": be, "ones_blk": ones_blk})
    return in_maps


def run(inputs, W, bias, trace=False, **spmd_kwargs):
    nc = _get_nc()
    in_maps = _prep_inputs(inputs, W, bias)
    res = run_bass_kernel_spmd(nc, in_maps, list(range(NCORES)),
                               trace=trace, **spmd_kwargs)
    v = np.concatenate([res.results[r]["out"] for r in range(NCORES)], axis=0)
    return v.reshape(B, J, D).astype(np.float32), res


def kernel(inputs, W, bias):
    out, _ = run(inputs, W, bias, trace=False)
    return out

